# revision 13
# baseline (speedup 1.0000x reference)
"""Trainium2 Bass/Tile kernel: 2-layer bidirectional LSTM encoder.

Contract: kernel(**inputs) takes the FULL unsharded inputs and returns the
full [T, B, 2H] fp32 output. Batch is split across 8 NeuronCores (data
parallel); weights replicated.

Shapes (hardcoded): T=160, B=256, C=512, H=256, G=4H=1024, 8 cores,
BC = 32 batch per core.

v2 design (per core, both directions fused per step):
 - Gate columns host-permuted to [i, g, f, o] with the g block pre-scaled
   by 2 so ONE sigmoid covers everything (tanh(x) = 2*sigmoid(2x) - 1).
 - xg = x @ Wih.T + bias precomputed per 4-step quad into PSUM (bf16
   matmuls, bias folded via K=1 ones-row matmul), evacuated to SBUF fp32
   by the otherwise-idle GPSIMD engine.
 - Per scan step, gates live in two PSUM banks gb0=[i|g], gb1=[f|o],
   each [64=(2dir,32b), 512]. xg is INJECTED into the banks with
   fp32r identity matmuls (start=True) - no DVE add on the critical path -
   then two K=128 bf16 recurrent matmuls per (dir, bank) accumulate.
 - ACT: sigmoid per bank -> bf16 SBUF. DVE (bf16 4x mode): ig via the
   2*s-1 fixup fused with scalar_tensor_tensor, c = f*c + ig, tanh(c) on
   ACT, h = o*tanh_c.
 - h [64, 256] bf16 -> PE transpose -> one DVE cast -> hT [128, 128]
   feeding the next step's recurrent matmuls. For layer 0, GPSIMD also
   copies the transposes into the h0T history that feeds layer 1's quads.
 - Layer-1 h rows DMA straight out as bf16; host upcasts to fp32.
"""

import os
import sys

import numpy as np

for _p in ("/opt/trn_rl_repo", "/root/.axon_site/_ro/trn_rl_repo"):
    if os.path.isdir(_p) and _p not in sys.path:
        sys.path.insert(0, _p)

from contextlib import ExitStack

import concourse.bass as bass  # noqa: F401
import concourse.mybir as mybir
import concourse.tile as tile
from concourse import bacc, bass_utils

AF = mybir.ActivationFunctionType
ALU = mybir.AluOpType
F32 = mybir.dt.float32
F32R = mybir.dt.float32r
BF16 = mybir.dt.bfloat16

T, B, CIN, H = 160, 256, 512, 256
G = 4 * H  # 1024
HALF = G // 2  # 512
Q = H  # 256 gate-block
NCORES = 8
BC = B // NCORES  # 32
NQ = T // 4  # 40 quads of 4 timesteps

# torch gate order [i,f,g,o] -> ours [i,g | f,o]; bank0=[i,g], bank1=[f,o]
_PERM = np.concatenate(
    [np.arange(0, 256), np.arange(512, 768), np.arange(256, 512),
     np.arange(768, 1024)]
)

_CACHE = {}


def _build():
    nc = bacc.Bacc("TRN2", target_bir_lowering=False, debug=False)

    xT_d = [
        nc.dram_tensor(f"xT{d}", [CIN, T * BC], BF16, kind="ExternalInput").ap()
        for d in (0, 1)
    ]
    wih_d = [
        [
            nc.dram_tensor(f"wih{l}{d}", [128, 4 * G], BF16, kind="ExternalInput").ap()
            for d in (0, 1)
        ]
        for l in (0, 1)
    ]
    whh_d = [
        [
            nc.dram_tensor(f"whh{l}{d}", [128, 2 * G], BF16, kind="ExternalInput").ap()
            for d in (0, 1)
        ]
        for l in (0, 1)
    ]
    bias_d = [
        [
            nc.dram_tensor(f"bias{l}{d}", [1, G], BF16, kind="ExternalInput").ap()
            for d in (0, 1)
        ]
        for l in (0, 1)
    ]
    ones_d = nc.dram_tensor("ones", [1, 128], BF16, kind="ExternalInput").ap()
    ident32_d = nc.dram_tensor("ident32", [128, 32], BF16, kind="ExternalInput").ap()
    identT_d = nc.dram_tensor("identT", [64, 64], BF16, kind="ExternalInput").ap()
    out_d = nc.dram_tensor("out", [T, BC, 2 * H], BF16, kind="ExternalOutput").ap()

    with tile.TileContext(nc) as tc, ExitStack() as ctx:
        sb = ctx.enter_context(tc.tile_pool(name="sb", bufs=2))
        const = ctx.enter_context(tc.tile_pool(name="const", bufs=1))
        big = ctx.enter_context(tc.tile_pool(name="big", bufs=1))
        ps_xg = ctx.enter_context(tc.tile_pool(name="ps_xg", bufs=1, space="PSUM"))
        ps_g = ctx.enter_context(tc.tile_pool(name="ps_g", bufs=2, space="PSUM"))
        ps_t = ctx.enter_context(tc.tile_pool(name="ps_t", bufs=2, space="PSUM"))

        identT_sb = const.tile([64, 64], BF16)
        nc.sync.dma_start(identT_sb[:], identT_d[:])
        ident32_sb = const.tile([128, 32], BF16)
        nc.sync.dma_start(ident32_sb[:], ident32_d[:])
        ones_sb = const.tile([1, 128], BF16)
        nc.sync.dma_start(ones_sb[:], ones_d[:])

        # h0T: layer-0 output history, feature-major:
        # [128, (k=2, dsrc=2, t=T, b=32)]  (k: h-feature chunk, dsrc: fwd/bwd)
        h0T = big.tile([128, 2 * T * 64], BF16)
        h0T_r = h0T[:].rearrange("p (k dd t b) -> p k dd t b", k=2, dd=2, t=T)

        for l in (0, 1):
            wih_sb = [
                sb.tile([128, 4 * G], BF16, tag=f"wih{d}", bufs=1, name=f"wih{l}{d}s")
                for d in (0, 1)
            ]
            whh_sb = [
                sb.tile([128, 2 * G], BF16, tag=f"whh{d}", bufs=1, name=f"whh{l}{d}s")
                for d in (0, 1)
            ]
            bias_sb = [
                sb.tile([1, G], BF16, tag=f"bias{d}", bufs=1, name=f"bias{l}{d}s")
                for d in (0, 1)
            ]
            for d in (0, 1):
                nc.sync.dma_start(wih_sb[d][:], wih_d[l][d][:])
                nc.sync.dma_start(whh_sb[d][:], whh_d[l][d][:])
                nc.sync.dma_start(bias_sb[d][:], bias_d[l][d][:])

            xg_live = {}

            gemm_ps = {}

            def emit_gemm_slice(q, d, half, l=l, wih_sb=wih_sb, bias_sb=bias_sb,
                                xg_live=xg_live, gemm_ps=gemm_ps):
                """Half of the xg GEMM for (quad q, dir d). half=0: ki 0-1
                (allocates the PSUM tile, start); half=1: ki 2-3 + bias
                (stop) + evacuation to SBUF bf16. Split so the PE gets an
                even stream of ready filler work every step."""
                if half == 0:
                    xg_ps = ps_xg.tile([128, G], F32, tag="xgps", name="xgps")
                    gemm_ps[(d, q)] = xg_ps
                else:
                    xg_ps = gemm_ps.pop((d, q))
                for ki in (0, 1) if half == 0 else (2, 3):
                    if l == 0:
                        stat = sb.tile([128, 128], BF16, tag="xstat", bufs=6)
                        nc.sync.dma_start(
                            stat[:],
                            xT_d[d][ki * 128 : (ki + 1) * 128, q * 128 : (q + 1) * 128],
                        )
                        lhsT = stat[:]
                    else:
                        t0 = 4 * q if d == 0 else T - 4 - 4 * q
                        k, dsrc = ki % 2, ki // 2
                        base = k * 10240 + dsrc * 5120 + t0 * 32
                        lhsT = h0T[:, base : base + 128]
                    for nh in (0, 1):
                        nc.tensor.matmul(
                            xg_ps[:, nh * HALF : (nh + 1) * HALF],
                            lhsT,
                            wih_sb[d][
                                :, ki * G + nh * HALF : ki * G + (nh + 1) * HALF
                            ],
                            start=(ki == 0),
                            stop=False,
                        )
                if half == 1:
                    for nh in (0, 1):
                        nc.tensor.matmul(
                            xg_ps[:, nh * HALF : (nh + 1) * HALF],
                            ones_sb[:],
                            bias_sb[d][:, nh * HALF : (nh + 1) * HALF],
                            start=False,
                            stop=True,
                        )
                    xg_t = sb.tile([128, G], BF16, tag=f"xg{d}", bufs=3)
                    nc.scalar.activation(xg_t[:, 0:HALF], xg_ps[:, 0:HALF], AF.Copy)
                    nc.vector.tensor_copy(xg_t[:, HALF:G], xg_ps[:, HALF:G])
                    xg_live[(d, q)] = xg_t

            def emit_gemm(q, d):
                emit_gemm_slice(q, d, 0)
                emit_gemm_slice(q, d, 1)

            def emit_inject(s, gb, d, l=l, xg_live=xg_live):
                """Inject xg for (step s, dir d) into its PSUM half via
                identity matmuls. start=True resets the written span; s==0
                also stops (no recurrent matmuls)."""
                q, r = divmod(s, 4)
                slot = r if (l == 0 or d == 0) else 3 - r
                src = xg_live[(d, q)]
                for nh in (0, 1):
                    nc.tensor.matmul(
                        gb[nh][32 * d : 32 * d + 32, :],
                        ident32_sb[32 * slot : 32 * slot + 32, :],
                        src[32 * slot : 32 * slot + 32,
                            nh * HALF : (nh + 1) * HALF],
                        start=True,
                        stop=(s == 0),
                        tile_position=(32 * slot, 32 * d),
                    )

            for q0 in (0, 1):
                emit_gemm(q0, 0)
                emit_gemm(q0, 1)

            # gates PSUM banks for step 0 + injects (both dirs)
            gb_cur = [
                ps_g.tile([64, HALF], F32, tag=f"gb{nh}", bufs=2, name=f"gb{nh}i")
                for nh in (0, 1)
            ]
            emit_inject(0, gb_cur, 0)
            emit_inject(0, gb_cur, 1)

            c_prev = [None, None]
            hT_prev = [None, None]
            for s in range(T):
                q, r = divmod(s, 4)

                gb = gb_cur
                if s + 1 < T:
                    gb_cur = [
                        ps_g.tile([64, HALF], F32, tag=f"gb{nh}", bufs=2,
                                  name=f"gb{nh}s")
                        for nh in (0, 1)
                    ]

                # PE: recurrents for both dirs first (bank0 before bank1),
                # then ready filler (gemm slice, next injects), then the
                # transposes (blocked on h, absorbed by the wait queue).
                if s > 0:
                    for nh in (0, 1):
                        for d in (0, 1):
                            grows = slice(32 * d, 32 * d + 32)
                            for k in (0, 1):
                                lhsT_h = hT_prev[d][:, k * 32 : k * 32 + 32]
                                nc.tensor.matmul(
                                    gb[nh][grows, :],
                                    lhsT_h,
                                    whh_sb[d][
                                        :,
                                        k * G + nh * HALF : k * G + (nh + 1) * HALF,
                                    ],
                                    start=False,
                                    stop=(k == 1),
                                    tile_position=(0, 32 * d),
                                )
                if q + 2 < NQ:
                    emit_gemm_slice(q + 2, r // 2, r % 2)
                if s + 1 < T:
                    emit_inject(s + 1, gb_cur, 0)
                    emit_inject(s + 1, gb_cur, 1)

                # ACT queue: all four sigmas, then the two tanh_c
                sig = [[None, None], [None, None]]
                for d in (0, 1):
                    for nh in (0, 1):
                        so = sb.tile([32, HALF], BF16, tag=f"s{d}{nh}", bufs=2,
                                     name=f"s{d}{nh}")
                        nc.scalar.activation(
                            so[:], gb[nh][32 * d : 32 * d + 32, :], AF.Sigmoid
                        )
                        sig[d][nh] = so

                # DVE cell math per dir (bf16); h + hT copy after tanh_c
                c_new = [None, None]
                for d in (0, 1):
                    s0, s1 = sig[d]
                    sgi = sb.tile([32, Q], BF16, tag=f"sgi{d}", name=f"sgi{d}")
                    nc.vector.tensor_mul(sgi[:], s0[:, Q:], s0[:, 0:Q])
                    ig = sb.tile([32, Q], BF16, tag=f"ig{d}", name=f"ig{d}")
                    nc.vector.scalar_tensor_tensor(
                        ig[:], sgi[:], 2.0, s0[:, 0:Q], ALU.mult, ALU.subtract
                    )
                    cn = sb.tile([32, Q], BF16, tag=f"c{d}", bufs=2, name=f"c{d}")
                    if s == 0:
                        nc.vector.tensor_copy(cn[:], ig[:])
                    else:
                        fc = sb.tile([32, Q], BF16, tag=f"fc{d}", name=f"fc{d}")
                        nc.vector.tensor_mul(fc[:], s1[:, 0:Q], c_prev[d][:])
                        nc.vector.tensor_add(cn[:], fc[:], ig[:])
                    c_prev[d] = cn
                    c_new[d] = cn

                tct = [None, None]
                for d in (0, 1):
                    tc = sb.tile([32, Q], BF16, tag=f"tct{d}", name=f"tct{d}")
                    nc.scalar.activation(tc[:], c_new[d][:], AF.Tanh)
                    tct[d] = tc

                t_f, t_b = s, T - 1 - s
                for d in (0, 1):
                    h = sb.tile([32, Q], BF16, tag=f"h{d}", bufs=3, name=f"h{d}")
                    nc.vector.tensor_mul(h[:], sig[d][1][:, Q:], tct[d][:])
                    if l == 1:
                        if d == 0:
                            nc.sync.dma_start(out_d[t_f, :, 0:256], h[:])
                        else:
                            nc.sync.dma_start(out_d[t_b, :, 256:512], h[:])
                    if l == 0 or s < T - 1:
                        trp = ps_t.tile([128, 64], BF16, tag=f"trp{d}", bufs=1,
                                        name=f"trp{d}")
                        for k in (0, 1):
                            nc.tensor.transpose(
                                trp[:, k * 32 : (k + 1) * 32],
                                h[:, k * 128 : (k + 1) * 128],
                                identT_sb[0:32, 0:32],
                            )
                        hT_new = sb.tile([128, 64], BF16, tag=f"hT{d}", bufs=2,
                                         name=f"hT{d}")
                        nc.vector.tensor_copy(hT_new[:], trp[:])
                        hT_prev[d] = hT_new
                        if l == 0:
                            td = t_f if d == 0 else t_b
                            for k in (0, 1):
                                nc.gpsimd.tensor_copy(
                                    h0T_r[:, k, d, td, :],
                                    hT_new[:, k * 32 : k * 32 + 32],
                                )

    nc.compile()
    return nc


def _prep_inputs(inputs):
    import ml_dtypes

    bf = ml_dtypes.bfloat16
    x = np.asarray(inputs["x"], dtype=np.float32)
    gscale = np.ones((G,), np.float32)
    gscale[256:512] = 2.0  # g block (post-perm) pre-doubled: tanh(x)=2sig(2x)-1
    common = {}
    for l in (0, 1):
        for d, sfx in enumerate(("", "_reverse")):
            Wih = np.asarray(inputs[f"weight_ih_l{l}{sfx}"], dtype=np.float32)
            Whh = np.asarray(inputs[f"weight_hh_l{l}{sfx}"], dtype=np.float32)
            bsum = (
                np.asarray(inputs[f"bias_ih_l{l}{sfx}"], dtype=np.float32)
                + np.asarray(inputs[f"bias_hh_l{l}{sfx}"], dtype=np.float32)
            )
            wihT = np.ascontiguousarray(Wih.T[:, _PERM]) * gscale  # [cin, 1024]
            whhT = np.ascontiguousarray(Whh.T[:, _PERM]) * gscale  # [256, 1024]
            common[f"wih{l}{d}"] = (
                wihT.reshape(4, 128, G).transpose(1, 0, 2).reshape(128, 4 * G)
            )
            common[f"whh{l}{d}"] = (
                whhT.reshape(2, 128, G).transpose(1, 0, 2).reshape(128, 2 * G)
            )
            common[f"bias{l}{d}"] = (bsum[_PERM] * gscale)[None, :]
    common["ones"] = np.ones((1, 128), np.float32)
    common["ident32"] = np.tile(np.eye(32, dtype=np.float32), (4, 1))
    common["identT"] = np.eye(64, dtype=np.float32)

    dts = {"identT": bf, "ident32": bf, "ones": bf}
    for l in (0, 1):
        for d in (0, 1):
            dts[f"bias{l}{d}"] = bf
    for l in (0, 1):
        for d in (0, 1):
            dts[f"wih{l}{d}"] = bf
            dts[f"whh{l}{d}"] = bf
    common = {
        k: np.ascontiguousarray(v, dtype=dts.get(k, np.float32))
        for k, v in common.items()
    }

    in_maps = []
    for c in range(NCORES):
        xs = x[:, c * BC : (c + 1) * BC, :]  # [T, 32, 512]
        m = dict(common)
        m["xT0"] = np.ascontiguousarray(
            xs.transpose(2, 0, 1).reshape(CIN, T * BC).astype(bf)
        )
        m["xT1"] = np.ascontiguousarray(
            xs[::-1].transpose(2, 0, 1).reshape(CIN, T * BC).astype(bf)
        )
        in_maps.append(m)
    return in_maps


def _get_program():
    if "prog" not in _CACHE:
        _CACHE["prog"] = _build()
    return _CACHE["prog"]


def kernel(**inputs):
    nc = _get_program()
    in_maps = _prep_inputs(inputs)
    res = bass_utils.run_bass_kernel_spmd(nc, in_maps, core_ids=list(range(NCORES)))
    out = np.empty((T, B, 2 * H), np.float32)
    for c in range(NCORES):
        out[:, c * BC : (c + 1) * BC, :] = np.asarray(
            res.results[c]["out"], dtype=np.float32
        )
    return out


# revision 15
# speedup vs baseline: 1.3250x; 1.3250x over previous
"""Trainium2 Bass/Tile kernel: 2-layer bidirectional LSTM encoder.

Contract: kernel(**inputs) takes the FULL unsharded inputs and returns the
full [T, B, 2H] fp32 output. Batch is split across 8 NeuronCores (data
parallel); weights replicated.

Shapes (hardcoded): T=160, B=256, C=512, H=256, G=4H=1024, 8 cores,
BC = 32 batch per core.

v2 design (per core, both directions fused per step):
 - Gate columns host-permuted to [i, g, f, o] with the g block pre-scaled
   by 2 so ONE sigmoid covers everything (tanh(x) = 2*sigmoid(2x) - 1).
 - xg = x @ Wih.T + bias precomputed per 4-step quad into PSUM (bf16
   matmuls, bias folded via K=1 ones-row matmul), evacuated to SBUF fp32
   by the otherwise-idle GPSIMD engine.
 - Per scan step, gates live in two PSUM banks gb0=[i|g], gb1=[f|o],
   each [64=(2dir,32b), 512]. xg is INJECTED into the banks with
   fp32r identity matmuls (start=True) - no DVE add on the critical path -
   then two K=128 bf16 recurrent matmuls per (dir, bank) accumulate.
 - ACT: sigmoid per bank -> bf16 SBUF. DVE (bf16 4x mode): ig via the
   2*s-1 fixup fused with scalar_tensor_tensor, c = f*c + ig, tanh(c) on
   ACT, h = o*tanh_c.
 - h [64, 256] bf16 -> PE transpose -> one DVE cast -> hT [128, 128]
   feeding the next step's recurrent matmuls. For layer 0, GPSIMD also
   copies the transposes into the h0T history that feeds layer 1's quads.
 - Layer-1 h rows DMA straight out as bf16; host upcasts to fp32.
"""

import os
import sys

import numpy as np

for _p in ("/opt/trn_rl_repo", "/root/.axon_site/_ro/trn_rl_repo"):
    if os.path.isdir(_p) and _p not in sys.path:
        sys.path.insert(0, _p)

from contextlib import ExitStack

import concourse.bass as bass  # noqa: F401
import concourse.mybir as mybir
import concourse.tile as tile
from concourse import bacc, bass_utils

AF = mybir.ActivationFunctionType
ALU = mybir.AluOpType
F32 = mybir.dt.float32
F32R = mybir.dt.float32r
BF16 = mybir.dt.bfloat16

T, B, CIN, H = 160, 256, 512, 256
G = 4 * H  # 1024
HALF = G // 2  # 512
Q = H  # 256 gate-block
NCORES = 8
BC = B // NCORES  # 32
NQ = T // 4  # 40 quads of 4 timesteps

# torch gate order [i,f,g,o] -> ours [i,g | f,o]; bank0=[i,g], bank1=[f,o]
_PERM = np.concatenate(
    [np.arange(0, 256), np.arange(512, 768), np.arange(256, 512),
     np.arange(768, 1024)]
)

_CACHE = {}


def _build():
    nc = bacc.Bacc("TRN2", target_bir_lowering=False, debug=False)

    xT_d = [
        nc.dram_tensor(f"xT{d}", [CIN, T * BC], BF16, kind="ExternalInput").ap()
        for d in (0, 1)
    ]
    wih_d = [
        [
            nc.dram_tensor(f"wih{l}{d}", [128, 4 * G], BF16, kind="ExternalInput").ap()
            for d in (0, 1)
        ]
        for l in (0, 1)
    ]
    whh_d = [
        [
            nc.dram_tensor(f"whh{l}{d}", [128, 2 * G], BF16, kind="ExternalInput").ap()
            for d in (0, 1)
        ]
        for l in (0, 1)
    ]
    bias_d = [
        [
            nc.dram_tensor(f"bias{l}{d}", [1, G], BF16, kind="ExternalInput").ap()
            for d in (0, 1)
        ]
        for l in (0, 1)
    ]
    ones_d = nc.dram_tensor("ones", [1, 128], BF16, kind="ExternalInput").ap()
    ident32_d = nc.dram_tensor("ident32", [128, 32], BF16, kind="ExternalInput").ap()
    identT_d = nc.dram_tensor("identT", [64, 64], BF16, kind="ExternalInput").ap()
    out_d = nc.dram_tensor("out", [T, BC, 2 * H], BF16, kind="ExternalOutput").ap()

    with tile.TileContext(nc) as tc, ExitStack() as ctx:
        sb = ctx.enter_context(tc.tile_pool(name="sb", bufs=2))
        const = ctx.enter_context(tc.tile_pool(name="const", bufs=1))
        big = ctx.enter_context(tc.tile_pool(name="big", bufs=1))
        ps_xg = ctx.enter_context(tc.tile_pool(name="ps_xg", bufs=1, space="PSUM"))
        ps_g = ctx.enter_context(tc.tile_pool(name="ps_g", bufs=2, space="PSUM"))
        ps_t = ctx.enter_context(tc.tile_pool(name="ps_t", bufs=2, space="PSUM"))

        identT_sb = const.tile([64, 64], BF16)
        nc.sync.dma_start(identT_sb[:], identT_d[:])
        ident32_sb = const.tile([128, 32], BF16)
        nc.sync.dma_start(ident32_sb[:], ident32_d[:])
        ones_sb = const.tile([1, 128], BF16)
        nc.sync.dma_start(ones_sb[:], ones_d[:])

        # h0T: layer-0 output history, feature-major:
        # [128, (k=2, dsrc=2, t=T, b=32)]  (k: h-feature chunk, dsrc: fwd/bwd)
        h0T = big.tile([128, 2 * T * 64], BF16)
        h0T_r = h0T[:].rearrange("p (k dd t b) -> p k dd t b", k=2, dd=2, t=T)

        for l in (0, 1):
            wih_sb = [
                sb.tile([128, 4 * G], BF16, tag=f"wih{d}", bufs=1, name=f"wih{l}{d}s")
                for d in (0, 1)
            ]
            whh_sb = [
                sb.tile([128, 2 * G], BF16, tag=f"whh{d}", bufs=1, name=f"whh{l}{d}s")
                for d in (0, 1)
            ]
            bias_sb = [
                sb.tile([1, G], BF16, tag=f"bias{d}", bufs=1, name=f"bias{l}{d}s")
                for d in (0, 1)
            ]
            for d in (0, 1):
                nc.sync.dma_start(wih_sb[d][:], wih_d[l][d][:])
                nc.sync.dma_start(whh_sb[d][:], whh_d[l][d][:])
                nc.sync.dma_start(bias_sb[d][:], bias_d[l][d][:])

            xg_live = {}

            gemm_ps = {}

            def emit_gemm_slice(q, d, half, l=l, wih_sb=wih_sb, bias_sb=bias_sb,
                                xg_live=xg_live, gemm_ps=gemm_ps):
                """Half of the xg GEMM for (quad q, dir d). half=0: ki 0-1
                (allocates the PSUM tile, start); half=1: ki 2-3 + bias
                (stop) + evacuation to SBUF bf16. Split so the PE gets an
                even stream of ready filler work every step."""
                if half == 0:
                    xg_ps = ps_xg.tile([128, G], F32, tag="xgps", name="xgps")
                    gemm_ps[(d, q)] = xg_ps
                else:
                    xg_ps = gemm_ps.pop((d, q))
                for ki in (0, 1) if half == 0 else (2, 3):
                    if l == 0:
                        stat = sb.tile([128, 128], BF16, tag="xstat", bufs=6)
                        nc.sync.dma_start(
                            stat[:],
                            xT_d[d][ki * 128 : (ki + 1) * 128, q * 128 : (q + 1) * 128],
                        )
                        lhsT = stat[:]
                    else:
                        t0 = 4 * q if d == 0 else T - 4 - 4 * q
                        k, dsrc = ki % 2, ki // 2
                        base = k * 10240 + dsrc * 5120 + t0 * 32
                        lhsT = h0T[:, base : base + 128]
                    for nq in range(4):
                        nc.tensor.matmul(
                            xg_ps[:, nq * Q : (nq + 1) * Q],
                            lhsT,
                            wih_sb[d][
                                :, ki * G + nq * Q : ki * G + (nq + 1) * Q
                            ],
                            start=(ki == 0 and nq % 2 == 0),
                            stop=False,
                            skip_group_check=True,
                        )
                if half == 1:
                    for nq in range(4):
                        nc.tensor.matmul(
                            xg_ps[:, nq * Q : (nq + 1) * Q],
                            ones_sb[:],
                            bias_sb[d][:, nq * Q : (nq + 1) * Q],
                            start=False,
                            stop=True,
                        )
                    xg_t = sb.tile([128, G], BF16, tag=f"xg{d}", bufs=3)
                    nc.scalar.activation(xg_t[:, 0:HALF], xg_ps[:, 0:HALF], AF.Copy)
                    nc.vector.tensor_copy(xg_t[:, HALF:G], xg_ps[:, HALF:G])
                    xg_live[(d, q)] = xg_t

            def emit_gemm(q, d):
                emit_gemm_slice(q, d, 0)
                emit_gemm_slice(q, d, 1)

            def emit_inject(s, gb, l=l, xg_live=xg_live):
                """Inject xg for step s into gates PSUM banks via fp32r
                identity matmuls. start=True resets the bank; s==0 also
                stops (no recurrent matmuls)."""
                q, r = divmod(s, 4)
                for d in (0, 1):
                    slot = r if (l == 0 or d == 0) else 3 - r
                    src = xg_live[(d, q)]
                    for nq in range(4):
                        nc.tensor.matmul(
                            gb[nq // 2][32 * d : 32 * d + 32,
                                        (nq % 2) * Q : (nq % 2 + 1) * Q],
                            ident32_sb[32 * slot : 32 * slot + 32, :],
                            src[32 * slot : 32 * slot + 32,
                                nq * Q : (nq + 1) * Q],
                            start=(nq % 2 == 0),
                            stop=(s == 0 and nq % 2 == 1),
                            tile_position=(32 * slot, 32 * d),
                            skip_group_check=True,
                        )

            for q0 in (0, 1):
                emit_gemm(q0, 0)
                emit_gemm(q0, 1)

            # gates PSUM banks for step 0 + inject
            gb_cur = [
                ps_g.tile([64, HALF], F32, tag=f"gb{nh}", bufs=2, name=f"gb{nh}i")
                for nh in (0, 1)
            ]
            emit_inject(0, gb_cur)

            c_prev = None
            hT_prev = None
            for s in range(T):
                q, r = divmod(s, 4)
                # spread future xg GEMM halves across steps (PE filler work)
                if q + 2 < NQ:
                    emit_gemm_slice(q + 2, r // 2, r % 2)

                gb = gb_cur
                # pre-allocate + inject NEXT step's banks (ready PE work that
                # fills the pipe while this step's recurrents wait on hT)
                if s + 1 < T:
                    gb_cur = [
                        ps_g.tile([64, HALF], F32, tag=f"gb{nh}", bufs=2,
                                  name=f"gb{nh}s")
                        for nh in (0, 1)
                    ]
                    emit_inject(s + 1, gb_cur)

                if s > 0:
                    # recurrent: gates += hT_prev.T @ Whh (bf16), bank0 first
                    for nh in (0, 1):
                        for d in (0, 1):
                            grows = slice(32 * d, 32 * d + 32)
                            for k in (0, 1):
                                lhsT_h = hT_prev[:, k * 64 + 32 * d : k * 64 + 32 * d + 32]
                                nc.tensor.matmul(
                                    gb[nh][grows, :],
                                    lhsT_h,
                                    whh_sb[d][
                                        :,
                                        k * G + nh * HALF : k * G + (nh + 1) * HALF,
                                    ],
                                    start=False,
                                    stop=(k == 1),
                                    tile_position=(0, 32 * d),
                                )

                # ACT: one sigmoid per bank -> bf16 SBUF
                s0 = sb.tile([64, HALF], BF16, tag="s0", bufs=2)  # [i | g']
                nc.scalar.activation(s0[:], gb[0][:], AF.Sigmoid)
                s1 = sb.tile([64, HALF], BF16, tag="s1", bufs=2)  # [f | o]
                nc.scalar.activation(s1[:], gb[1][:], AF.Sigmoid)

                # DVE cell math, bf16 in SBUF (4x mode)
                # g = 2*s_g - 1 (tanh via sigmoid); ig = i*g = 2*(s_g*i) - i
                sgi = sb.tile([64, Q], BF16, tag="sgi")
                nc.vector.tensor_mul(sgi[:], s0[:, Q:], s0[:, 0:Q])
                ig = sb.tile([64, Q], BF16, tag="ig")
                nc.vector.scalar_tensor_tensor(
                    ig[:], sgi[:], 2.0, s0[:, 0:Q], ALU.mult, ALU.subtract
                )
                c_new = sb.tile([64, Q], BF16, tag="c", bufs=2)
                if s == 0:
                    nc.vector.tensor_copy(c_new[:], ig[:])
                else:
                    fc = sb.tile([64, Q], BF16, tag="fc")
                    nc.vector.tensor_mul(fc[:], s1[:, 0:Q], c_prev[:])
                    nc.vector.tensor_add(c_new[:], fc[:], ig[:])
                c_prev = c_new
                tct = sb.tile([64, Q], BF16, tag="tct")
                nc.scalar.activation(tct[:], c_new[:], AF.Tanh)
                h = sb.tile([64, Q], BF16, tag="h", bufs=3)
                nc.vector.tensor_mul(h[:], s1[:, Q:], tct[:])

                t_f, t_b = s, T - 1 - s
                if l == 1:
                    nc.sync.dma_start(out_d[t_f, :, 0:256], h[0:32, :])
                    nc.sync.dma_start(out_d[t_b, :, 256:512], h[32:64, :])

                if l == 0 or s < T - 1:
                    # hT for next step's recurrents: PE transpose + 1 DVE cast
                    trp = ps_t.tile([128, 128], BF16, tag="trp", bufs=2)
                    for k in (0, 1):
                        nc.tensor.transpose(
                            trp[:, k * 64 : (k + 1) * 64],
                            h[:, k * 128 : (k + 1) * 128],
                            identT_sb[:],
                        )
                    hT_new = sb.tile([128, 128], BF16, tag="hT", bufs=2)
                    nc.vector.tensor_copy(hT_new[:], trp[:])
                    hT_prev = hT_new
                    if l == 0:
                        # history writes for layer 1 (off critical path, GPSIMD)
                        for k in (0, 1):
                            nc.gpsimd.tensor_copy(
                                h0T_r[:, k, 0, t_f, :],
                                hT_new[:, k * 64 : k * 64 + 32],
                            )
                            nc.gpsimd.tensor_copy(
                                h0T_r[:, k, 1, t_b, :],
                                hT_new[:, k * 64 + 32 : k * 64 + 64],
                            )

    nc.compile()
    return nc


def _prep_inputs(inputs):
    import ml_dtypes

    bf = ml_dtypes.bfloat16
    x = np.asarray(inputs["x"], dtype=np.float32)
    gscale = np.ones((G,), np.float32)
    gscale[256:512] = 2.0  # g block (post-perm) pre-doubled: tanh(x)=2sig(2x)-1
    common = {}
    for l in (0, 1):
        for d, sfx in enumerate(("", "_reverse")):
            Wih = np.asarray(inputs[f"weight_ih_l{l}{sfx}"], dtype=np.float32)
            Whh = np.asarray(inputs[f"weight_hh_l{l}{sfx}"], dtype=np.float32)
            bsum = (
                np.asarray(inputs[f"bias_ih_l{l}{sfx}"], dtype=np.float32)
                + np.asarray(inputs[f"bias_hh_l{l}{sfx}"], dtype=np.float32)
            )
            wihT = np.ascontiguousarray(Wih.T[:, _PERM]) * gscale  # [cin, 1024]
            whhT = np.ascontiguousarray(Whh.T[:, _PERM]) * gscale  # [256, 1024]
            common[f"wih{l}{d}"] = (
                wihT.reshape(4, 128, G).transpose(1, 0, 2).reshape(128, 4 * G)
            )
            common[f"whh{l}{d}"] = (
                whhT.reshape(2, 128, G).transpose(1, 0, 2).reshape(128, 2 * G)
            )
            common[f"bias{l}{d}"] = (bsum[_PERM] * gscale)[None, :]
    common["ones"] = np.ones((1, 128), np.float32)
    common["ident32"] = np.tile(np.eye(32, dtype=np.float32), (4, 1))
    common["identT"] = np.eye(64, dtype=np.float32)

    dts = {"identT": bf, "ident32": bf, "ones": bf}
    for l in (0, 1):
        for d in (0, 1):
            dts[f"bias{l}{d}"] = bf
    for l in (0, 1):
        for d in (0, 1):
            dts[f"wih{l}{d}"] = bf
            dts[f"whh{l}{d}"] = bf
    common = {
        k: np.ascontiguousarray(v, dtype=dts.get(k, np.float32))
        for k, v in common.items()
    }

    in_maps = []
    for c in range(NCORES):
        xs = x[:, c * BC : (c + 1) * BC, :]  # [T, 32, 512]
        m = dict(common)
        m["xT0"] = np.ascontiguousarray(
            xs.transpose(2, 0, 1).reshape(CIN, T * BC).astype(bf)
        )
        m["xT1"] = np.ascontiguousarray(
            xs[::-1].transpose(2, 0, 1).reshape(CIN, T * BC).astype(bf)
        )
        in_maps.append(m)
    return in_maps


def _get_program():
    if "prog" not in _CACHE:
        _CACHE["prog"] = _build()
    return _CACHE["prog"]


def kernel(**inputs):
    nc = _get_program()
    in_maps = _prep_inputs(inputs)
    res = bass_utils.run_bass_kernel_spmd(nc, in_maps, core_ids=list(range(NCORES)))
    out = np.empty((T, B, 2 * H), np.float32)
    for c in range(NCORES):
        out[:, c * BC : (c + 1) * BC, :] = np.asarray(
            res.results[c]["out"], dtype=np.float32
        )
    return out
